# revision 40
# baseline (speedup 1.0000x reference)
"""Trainium2 Bass kernel for DGMG AddEdge log-prob (gnn_message_passing).

Math restructure (exact in real arithmetic):
    gate  = sigmoid(hv @ Wg + bg)                 per node
    hdotc = hv @ (Wp @ We_g) + (bp @ We_g)        per node (head folded through
            the projection; segment_sum commutes with the dot)
    logit = sum_{n in g} gate*hdotc + hv[last_idx] @ We_s + be
    out   = logsigmoid((2a - 1) * logit) = -log1p(exp(-(2a-1)*logit))
The [B, G] graph embedding is never materialized.

Layout: graphs are packed into 8 blocks of 128 graphs per core; each block's
nodes are padded to 8 load tiles of 1024 nodes, so block <-> tile mapping is
static.  hv streams in TRANSPOSED tiles [128 feat, 1024 nodes] over all three
DMA-capable paths: SP + ACT issue f16 HWDGE copies, POOL issues f32 SWDGE
row-gathers (identity indices) whose completion latency is far lower.  Per
128-node window the PE computes [nodes, 2] = hvT^T @ [-Wg | w1]; ACT
exponentiates -glog, DVE forms ghd = hdotc / (1 + e) (== gate*hdotc), and
4-wide window one-hot GEMVs reduce to per-(window,slot) partial sums vdP.
Per block, a transpose + 4 one-hot P matmuls + one src-embedding matmul
assemble logits straight into PSUM [128 graphs, 1]; Exp/Ln(bias=1) give the
log-prob, and a SWDGE scatter-add writes the output rows (cheap tail).
"""
import os
import sys

import numpy as np

for _p in ("/opt/trn_rl_repo",):
    if os.path.isdir(_p) and _p not in sys.path:
        sys.path.insert(0, _p)

import concourse.bass as bass
import concourse.mybir as mybir
import concourse.tile as tile
from concourse import library_config
from concourse.bass_utils import run_bass_kernel_spmd
from concourse.library_overlay import lower_extended_insts

F32 = mybir.dt.float32
F16 = mybir.dt.float16
I16 = mybir.dt.int16
AL = mybir.AluOpType
AF = mybir.ActivationFunctionType

NCORES = 8
N, B, D = 500_000, 8192, 128
BL = B // NCORES           # graphs per core
NGB = 8                    # graph blocks per core
GBG = BL // NGB            # 128 graphs per block
TPB = 8                    # load tiles per block
NT = NGB * TPB             # 64 load tiles per core
TIL = 128                  # nodes per window tile (= feature count)
TILB = 1024                # nodes per load tile
HGRP = TILB // TIL         # 8 window groups per load tile
WPB = TPB * HGRP           # 64 window tiles per block
NW = NGB * WPB             # 512 window tiles per core
S = 4                      # window slots per 128-node group
VW = 64                    # out row width (f32) = 256B for dma_scatter_add
PAD_SEGREL = 99.0

# cst (f16) column layout; chunk A (before CA_END) loads first.
# All matmul weight reads stay at base partition 0 over 64 partitions, so
# the src contraction is split into lo/hi feature halves (hi repacked to
# rows 0..63) and P lives entirely in rows 0..63.
C_SEG = 0                          # seg_t [128, NW]
C_IOT = C_SEG + NW                 # iota slot pattern [128, 256]
C_W = C_IOT + TIL * 2              # [-Wg | w1] f16
C_WES = C_W + 2                    # wes[0:64] in rows 0..63
C_WES2 = C_WES + 1                 # wes[64:128] repacked to rows 0..63
CA_END = C_WES2 + 1                # early chunk end
C_SRC = CA_END                     # srcT [128, 1024] (rows = features)
C_P = C_SRC + BL                   # P one-hot [64 rows, 32*128]
C_SRC2 = C_P + 32 * GBG            # srcT[64:128] repacked to rows 0..63
C_TOT = C_SRC2 + BL
# cst32 (f32) column layout: sgn_neg [128, 8] | [-Wg | w1] f32 | eye(4)
C32_SGN, C32_W = 0, NGB
C32_IDF = C32_W + 2
C32_TOT = C32_IDF + S

LAST_RESULTS = None
LAST_NC = None

_WS_CTR = [0]


def split_sync_waits(nc, maxw=1):
    """This walrus build rejects instructions with more than one semaphore
    wait; hoist excess waits onto injected same-engine NoOps."""
    for fn in nc.m.functions:
        for bb in fn.blocks:
            out, changed = [], False
            for inst in bb.instructions:
                si = inst.sync_info
                if si is not None and si.on_wait and len(si.on_wait) > maxw:
                    SI = type(si)
                    waits = list(si.on_wait)
                    extra, keep = waits[:-maxw], waits[-maxw:]
                    for k in range(0, len(extra), maxw):
                        nop = mybir.InstNoOp(
                            name=f"waitsplit_{_WS_CTR[0]}", ins=[], outs=[])
                        _WS_CTR[0] += 1
                        nop.engine = inst.engine
                        nop.bass_nofuse = True
                        nop.sync_info = SI(
                            on_wait=extra[k:k + maxw], on_update=[])
                        out.append(nop)
                    inst.sync_info = SI(
                        on_wait=keep, on_update=list(si.on_update or []))
                    changed = True
                out.append(inst)
            if changed:
                bb.instructions = out
    return nc


def _plan_queues(g_ext):
    """Greedy finish-time balance of the 64 hv tiles over the 3 DMA paths.
    Returns qs[t] in {"sp", "act", "pool"}.  Pool's first 3 tiles become
    SWDGE copies (the gather idx table is still in flight); gb7's last
    tiles are pinned to pool (low-latency gathers shorten the tail)."""
    load = {"sp": 2600.0, "act": 6800.0, "pool": 450.0}
    cost = {"sp": 98.7, "act": 98.7, "pool": 106.6}
    over = {"sp": 30.0, "act": 30.0, "pool": 40.0}
    qs = [None] * NT
    for t in range(NT):
        if t >= NT - 3:
            e = "pool"                  # tail tiles: cheap-latency gathers
        else:
            e = min(load, key=lambda k: load[k] + cost[k] * 8)
        load[e] += cost[e] * g_ext[t] + over[e]
        qs[t] = e
    return qs


def _build(g_ext, bg0, be0, c1):
    nc = bass.Bass()
    hv16_d = nc.declare_dram_parameter("hv16", [NT, TIL, TILB], F16,
                                       isOutput=False)
    hv32_d = nc.declare_dram_parameter("hv32", [NT, TIL, TILB], F32,
                                       isOutput=False)
    cst_d = nc.declare_dram_parameter("cst", [TIL, C_TOT], F16, isOutput=False)
    cst32_d = nc.declare_dram_parameter("cst32", [TIL, C32_TOT], F32,
                                        isOutput=False)
    i16_d = nc.declare_dram_parameter("i16o", [TIL, BL // 16], I16,
                                      isOutput=False)
    out_d = nc.declare_dram_parameter("out", [BL, VW], F32, isOutput=True)

    qs = _plan_queues(g_ext)
    # pool's first 3 assigned tiles go through SWDGE copies (idx in flight)
    pool_seen = 0
    mode = [None] * NT                 # "c16" = f16 copy, "g32" = f32 gather
    for t in range(NT):
        if qs[t] == "pool":
            mode[t] = "c16" if pool_seen < 3 else "g32"
            pool_seen += 1
        else:
            mode[t] = "c16"

    with tile.TileContext(nc) as tc:
        with (
            tc.tile_pool(name="consts", bufs=1) as cpool,
            tc.tile_pool(name="hv16p", bufs=12) as h16pool,
            tc.tile_pool(name="hv32p", bufs=8) as h32pool,
            tc.tile_pool(name="small", bufs=6) as spool,
            tc.tile_pool(name="pgh", bufs=2, space="PSUM") as ghpool,
            tc.tile_pool(name="pvd", bufs=3, space="PSUM") as vdpool,
            tc.tile_pool(name="pvt", bufs=1, space="PSUM") as vtpool,
            tc.tile_pool(name="plg", bufs=2, space="PSUM") as lgpool,
        ):
            nc.gpsimd.load_library(library_config.mlp)
            # --- constants ---
            cst = cpool.tile([TIL, C_TOT], F16)
            nc.sync.dma_start(cst[:, 0:CA_END], cst_d[:, 0:CA_END])
            # srcT + P ride at the head of ACT's queue (its activation work
            # only starts once block 0's dots land, ~4.5us in)
            nc.scalar.dma_start(cst[:, C_SRC:C_P], cst_d[:, C_SRC:C_P])
            nc.scalar.dma_start(cst[0:WPB, C_P:], cst_d[0:WPB, C_P:])
            cst32 = cpool.tile([TIL, C32_TOT], F32)
            nc.sync.dma_start(cst32[:], cst32_d[:])
            i16o = cpool.tile([TIL, BL // 16], I16)
            nc.sync.dma_start(i16o[:], i16_d[:])

            seg_t = cst[:, C_SEG:C_SEG + NW]
            iot = cst[:, C_IOT:C_IOT + TIL * 2]
            w16 = cst[:, C_W:C_W + 2]
            wes_lo = cst[0:WPB, C_WES:C_WES + 1]
            wes_hi = cst[0:WPB, C_WES2:C_WES2 + 1]
            idf = cst32[:, C32_IDF:C32_IDF + S]
            w32 = cst32[:, C32_W:C32_W + 2]
            sgn = cst32[:, C32_SGN:C32_SGN + NGB]

            # window one-hot slot masks, all blocks upfront (DVE idles early)
            sels = []
            for gb in range(NGB):
                sel = cpool.tile([TIL, WPB * S], F16, name=f"sel{gb}")
                nc.vector.tensor_tensor(
                    out=sel[:].rearrange("p (x j) -> p x j", j=S),
                    in0=seg_t[:, WPB * gb:WPB * (gb + 1)].rearrange(
                        "p (x one) -> p x one", one=1
                    ).to_broadcast([TIL, WPB, S]),
                    in1=iot[:, 0:WPB * S].rearrange("p (x j) -> p x j", j=S),
                    op=AL.is_equal)
                sels.append(sel)

            xall = cpool.tile([TIL, NGB], F32, name="xall")

            ghPs = [None] * NGB
            hvts = [None] * NGB      # list of (tile, is_f32) per load tile

            def emit_stream(gb):
                """DMAs + per-node dot matmuls for block gb."""
                ghP = ghpool.tile([TIL, 2 * WPB], F32, name="ghP")
                ghPs[gb] = ghP
                tl = []
                for i in range(TPB):
                    t = TPB * gb + i
                    g = HGRP        # full tiles: DRAM pad is zeros
                    if mode[t] == "g32":
                        hv1 = h32pool.tile([TIL, 1, TILB], F32, name="hv32t")
                        nc.gpsimd.dma_gather(
                            out_ap=hv1[:, :, 0:TIL * g],
                            in_ap=hv32_d[t][:, 0:TIL * g],
                            idxs_ap=i16o[:, 0:8],
                            num_idxs=TIL,
                            num_idxs_reg=TIL,
                            elem_size=TIL * g,
                            elem_step=TILB)
                        tl.append((hv1, True))
                    else:
                        hv1 = h16pool.tile([TIL, TILB], F16, name="hv16t")
                        eng = {"sp": nc.sync, "act": nc.scalar,
                               "pool": nc.gpsimd}[qs[t]]
                        eng.dma_start(hv1[:, 0:TIL * g],
                                      hv16_d[t][:, 0:TIL * g])
                        tl.append((hv1, False))
                    for g8 in range(HGRP):
                        col = 2 * (HGRP * i + g8)
                        if tl[-1][1]:
                            lhsT = hv1[:, 0, TIL * g8:TIL * (g8 + 1)]
                            rhs = w32
                        else:
                            lhsT = hv1[:, TIL * g8:TIL * (g8 + 1)]
                            rhs = w16
                        nc.tensor.matmul(ghP[:, col:col + 2], lhsT=lhsT,
                                         rhs=rhs, start=True, stop=True)
                hvts[gb] = tl

            def emit_compute(gb):
                """gate/ghd, window GEMVs, logit assembly for block gb."""
                ghP = ghPs[gb]
                ghv = ghP[:].rearrange("p (x two) -> p x two", two=2)
                e1 = spool.tile([TIL, WPB], F32, name="e1")
                nc.scalar.activation(e1[:], ghv[:, :, 0], AF.Exp, bias=-bg0)
                t1 = spool.tile([TIL, WPB], F32, name="t1")
                nc.vector.tensor_scalar_add(t1[:], e1[:], 1.0)
                r1 = spool.tile([TIL, WPB], F32, name="r1")
                nc.vector.reciprocal_approx_fast(r1[:], t1[:])
                ghd = spool.tile([TIL, WPB], F16, name="ghd")
                hdc_in = ghv[:, :, 1]
                if c1 != 0.0:
                    hdcb = spool.tile([TIL, WPB], F32, name="hdcb")
                    nc.vector.tensor_scalar_add(hdcb[:], hdc_in, c1)
                    hdc_in = hdcb[:]
                nc.vector.tensor_tensor(out=ghd[:], in0=hdc_in, in1=r1[:],
                                        op=AL.mult)
                sel = sels[gb]
                vdP = vdpool.tile([S, WPB], F32, name="vdP")
                for w in range(WPB):
                    nc.tensor.matmul(
                        vdP[0:S, w:w + 1],
                        lhsT=sel[:, S * w:S * (w + 1)],
                        rhs=ghd[:, w:w + 1], start=True, stop=True)
                vstg = spool.tile([S, WPB], F32, name="vstg")
                nc.vector.tensor_copy(vstg[:], vdP[0:S, :])
                vTP = vtpool.tile([WPB, S], F32, name="vTP")
                nc.tensor.transpose(vTP[:], vstg[0:S, :], idf[0:S, :])
                vTs = spool.tile([WPB, S], F16, name="vTs")
                nc.vector.tensor_copy(vTs[:], vTP[:])
                # NEFF-path constraints (found empirically): every matmul in
                # one accumulation group must contract the same 64 partitions
                # at base 0 — mixed 128/64 groups and offset-64 weight reads
                # both miscompute there
                lgP = lgpool.tile([TIL, 1], F32, name="lgP")
                nc.tensor.matmul(lgP[:], lhsT=cst[0:WPB, C_SRC + GBG * gb:
                                                  C_SRC + GBG * (gb + 1)],
                                 rhs=wes_lo, start=True, stop=False)
                nc.tensor.matmul(lgP[:], lhsT=cst[0:WPB, C_SRC2 + GBG * gb:
                                                  C_SRC2 + GBG * (gb + 1)],
                                 rhs=wes_hi, start=False, stop=False)
                for j in range(S):
                    q = S * gb + j
                    nc.tensor.matmul(
                        lgP[:], lhsT=cst[0:WPB,
                                         C_P + GBG * q:C_P + GBG * (q + 1)],
                        rhs=vTs[:, j:j + 1],
                        start=False, stop=(j == S - 1))
                nc.vector.tensor_tensor(out=xall[:, gb:gb + 1], in0=lgP[:],
                                        in1=sgn[:, gb:gb + 1], op=AL.mult)

            def emit_out(c0, c1b):
                """-log1p(exp(x')) for blocks [c0, c1b) + DMA to out rows."""
                k = c1b - c0
                eb = spool.tile([TIL, NGB], F32, name="eb")
                nc.scalar.activation(eb[:, 0:k], xall[:, c0:c1b], AF.Exp)
                lp = spool.tile([TIL, NGB], F32, name="lp")
                nc.scalar.activation(lp[:, 0:k], eb[:, 0:k], AF.Ln, bias=1.0)
                ob = spool.tile([TIL, NGB], F32, name="ob")
                nc.vector.tensor_scalar_mul(ob[:, 0:k], lp[:, 0:k], -1.0)
                nc.sync.dma_start(
                    out_d[GBG * c0:GBG * c1b, 0:1].rearrange(
                        "(c p) w -> p (c w)", p=TIL),
                    ob[:, 0:k])

            # one-block-deep software pipeline: stream gb+1, then compute gb
            emit_stream(0)
            for gb in range(1, NGB):
                emit_stream(gb)
                emit_compute(gb - 1)
                if gb - 1 == 5:
                    emit_out(0, 6)
            emit_compute(NGB - 1)
            emit_out(6, 7)
            emit_out(7, 8)
    return nc


def _prep_core(hv, hv16g, seg_ids, last_idx, a, m):
    """Per-core host prep: block packing, masks, P one-hot, src, sgn."""
    lo = int(np.searchsorted(seg_ids, m * BL, "left"))
    hi = int(np.searchsorted(seg_ids, (m + 1) * BL, "left"))
    seg_loc = seg_ids[lo:hi].astype(np.int64) - m * BL

    hv32 = np.zeros((NT, TIL, TILB), np.float32)
    hv16 = np.zeros((NT, TIL, TILB), np.float16)
    segrel = np.full((NGB, TPB * TILB), PAD_SEGREL, np.float32)
    P = np.zeros((WPB, 32 * GBG), np.float16)
    g_live = np.zeros(NT, np.int64)

    for gb in range(NGB):
        a0 = int(np.searchsorted(seg_loc, GBG * gb, "left"))
        b0 = int(np.searchsorted(seg_loc, GBG * (gb + 1), "left"))
        n = b0 - a0
        assert n <= TPB * TILB, f"core {m} block {gb}: {n} nodes > capacity"
        nodes = seg_loc[a0:b0]                       # global graph ids
        nrel = nodes - GBG * gb                      # 0..127 within block
        hvb = hv[lo + a0:lo + b0]                    # [n, D] f32
        blk32 = np.zeros((TPB * TILB, D), np.float32)
        blk32[:n] = hvb
        t0 = TPB * gb
        hv32[t0:t0 + TPB] = blk32.reshape(TPB, TILB, D).transpose(0, 2, 1)
        hv16[t0:t0 + TPB] = hv32[t0:t0 + TPB].astype(np.float16)
        for i in range(TPB):
            live = min(max(n - TILB * i, 0), TILB)
            g_live[t0 + i] = (live + TIL - 1) // TIL

        # window bases and per-node relative slot
        nwt = (n + TIL - 1) // TIL                   # live window tiles
        bT = np.zeros(WPB, np.int64)
        if nwt:
            bT[:nwt] = nodes[np.arange(nwt) * TIL]
        rel = nodes - bT[np.arange(n) // TIL]
        assert rel.min() >= 0 and rel.max() < S, "window slot overflow"
        segrel[gb, :n] = rel

        # home/spill per graph -> P one-hot
        rr = np.arange(GBG, dtype=np.int64) + GBG * gb
        firsts = np.searchsorted(nodes, rr, "left")
        lasts = np.searchsorted(nodes, rr + 1, "left")
        nonempty = firsts < lasts
        th = firsts // TIL
        tlast = np.maximum(lasts - 1, 0) // TIL
        assert np.all((tlast - th)[nonempty] <= 1), "segment spans > 2 tiles"
        j1 = rr - bT[np.clip(th, 0, WPB - 1)]
        straddle = nonempty & (tlast > th)
        assert np.all(bT[tlast[straddle]] == rr[straddle])
        for g in range(GBG):
            if not nonempty[g]:
                continue
            q = S * gb + int(j1[g])
            P[int(th[g]), GBG * q + g] = 1.0
            if straddle[g]:
                P[int(tlast[g]), GBG * S * gb + g] += 1.0

    seg_t = np.ascontiguousarray(
        segrel.reshape(NGB, WPB, TIL).transpose(2, 0, 1).reshape(TIL, NW)
    ).astype(np.float16)
    src = hv16g[last_idx[m * BL:(m + 1) * BL]]       # [1024, 128] f16
    srcT = np.ascontiguousarray(src.T)               # [128, 1024]
    sgn = -(2 * a[m * BL:(m + 1) * BL] - 1).astype(np.float32)
    sgn_p = np.ascontiguousarray(sgn.reshape(NGB, GBG).T)   # [128, 8]
    return hv32, hv16, seg_t, P, srcT, sgn_p, g_live


def prep_all(hv, Wg, bg, Wp, bp, We, be, seg_ids, last_idx, a):
    hv = np.asarray(hv, dtype=np.float32)
    Wg = np.asarray(Wg, dtype=np.float32)
    bg = np.asarray(bg, dtype=np.float32)
    Wp = np.asarray(Wp, dtype=np.float32)
    bp = np.asarray(bp, dtype=np.float32)
    We = np.asarray(We, dtype=np.float32)
    be = np.asarray(be, dtype=np.float32)
    seg_ids = np.asarray(seg_ids)
    last_idx = np.asarray(last_idx)
    a = np.asarray(a)

    G = 2 * D
    w1 = (Wp @ We[:G]).astype(np.float32)[:, 0]
    wes = We[G:, 0].astype(np.float32)
    c1 = float(bp @ We[:G, 0])
    bg0, be0 = float(bg[0]), float(be[0])
    assert be0 == 0.0, "be != 0 not folded in this build"

    slot = np.arange(S, dtype=np.float32)
    iot = np.tile(slot, TIL * 2 // S)[None, :].repeat(TIL, 0)

    # identity idx table [p%16 + 16c], wrapped i16 layout for SWDGE
    cols = BL // 16
    i16o = ((np.arange(TIL)[:, None] % 16) +
            16 * np.arange(cols)[None, :]).astype(np.int16)

    hv16g = hv.astype(np.float16)
    in_maps = []
    g_ext = np.ones(NT, np.int64)
    cores = []
    for m in range(NCORES):
        cores.append(_prep_core(hv, hv16g, seg_ids, last_idx, a, m))
        g_ext = np.maximum(g_ext, cores[-1][6])

    for m in range(NCORES):
        hv32, hv16, seg_t, P, srcT, sgn_p, _ = cores[m]
        cst = np.zeros((TIL, C_TOT), np.float16)
        cst[:, C_SEG:C_SEG + NW] = seg_t
        cst[:, C_IOT:C_IOT + TIL * 2] = iot
        cst[0:WPB, C_P:C_P + 32 * GBG] = P
        cst[:, C_SRC:C_SRC + BL] = srcT
        cst[0:WPB, C_SRC2:C_SRC2 + BL] = srcT[WPB:TIL, :]
        cst[:, C_W] = -Wg[:, 0]
        cst[:, C_W + 1] = w1
        cst[0:WPB, C_WES] = wes[0:WPB]
        cst[0:WPB, C_WES2] = wes[WPB:TIL]
        cst32 = np.zeros((TIL, C32_TOT), np.float32)
        cst32[:, C32_SGN:C32_SGN + NGB] = sgn_p
        cst32[:, C32_W] = -Wg[:, 0]
        cst32[:, C32_W + 1] = w1
        cst32[0:S, C32_IDF:C32_IDF + S] = np.eye(S, dtype=np.float32)
        in_maps.append({
            "hv16": hv16, "hv32": hv32,
            "cst": np.ascontiguousarray(cst),
            "cst32": cst32, "i16o": i16o,
        })
    return in_maps, [int(v) for v in g_ext], bg0, be0, c1


def kernel(hv, Wg, bg, Wp, bp, We, be, seg_ids, last_idx, a):
    global LAST_RESULTS, LAST_NC
    in_maps, g_ext, bg0, be0, c1 = prep_all(
        hv, Wg, bg, Wp, bp, We, be, seg_ids, last_idx, a)
    nc = _build(g_ext, bg0, be0, c1)
    split_sync_waits(nc, maxw=1)
    lower_extended_insts(nc)
    LAST_NC = nc
    res = run_bass_kernel_spmd(nc, in_maps, core_ids=list(range(NCORES)))
    LAST_RESULTS = res
    out = np.concatenate(
        [np.asarray(res.results[i]["out"])[:, 0:1] for i in range(NCORES)],
        axis=0)
    return out.astype(np.float32)
